# revision 8
# baseline (speedup 1.0000x reference)
"""Trainium2 Bass kernel for nn_Attn: attn = softmax(enc @ W^T @ hidden^T).

Math: reference computes energy = enc @ W^T + b  ([S,H]), then
attn_energies = energy @ hidden[0]  ([S]), then softmax over S.
Associativity: attn_energies = enc @ (W^T @ hidden^T) + (b . hidden).
The (b . hidden) term is a constant shift over S -> softmax-invariant
(and b is zeros for this problem), so we drop it.

Distribution over 8 cores:
  - enc sharded by sequence: core r owns rows [r*2048, (r+1)*2048).
  - W sharded by output column: core r owns W[:, r*256:(r+1)*256],
    computes u_r = hidden @ W[:, shard] on the PE, AllGather -> u [2048].
  - e_local = enc_shard @ u via one fused DVE multiply+row-sum per
    [128, 2048] tile (scalar_tensor_tensor with accum_out).
  - AllGather e -> every core holds all 16384 energies; each does the
    softmax redundantly and writes the full [16384] result; host takes
    core 0's copy.  Cross-partition max via PE transpose + row reduce,
    cross-partition sum via matmul with a ones vector, scalar
    broadcasts via rank-1 matmul.
"""

import numpy as np

S = 16384
H = 2048
NCORES = 8
S_LOC = S // NCORES  # 2048 sequence rows per core
H_SH = H // NCORES  # 256 W columns per core
P = 128
NT = S_LOC // P  # 16 enc tiles of [128, 2048] per core
NO = H // P  # 16 contraction chunks for the u matvec

_CACHE = {}


def _build_program():
    import concourse.bacc as bacc
    import concourse.mybir as mybir
    import concourse.tile as tile

    fp32 = mybir.dt.float32
    # Bacc (not raw Bass): its compile() splits multi-sem waits into
    # EventSemaphores and moves matmul waits onto ldweights -- TRN2
    # instructions carry at most one sync wait.
    nc = bacc.Bacc("TRN2")

    enc_in = nc.dram_tensor("enc", [S_LOC, H], fp32, kind="ExternalInput")
    # packed per-core weights: wh[p, o, 0:H_SH] = W[o*128+p, shard],
    # wh[p, o, H_SH] = hidden[o*128+p] -- one DMA so the first matmul
    # carries a single sync wait (LDWEIGHTS has one wait slot).
    wh_in = nc.dram_tensor("wh", [P, NO, H_SH + 1], fp32, kind="ExternalInput")
    attn_out = nc.dram_tensor("attn", [S], fp32, kind="ExternalOutput")

    ident_dram = nc.inline_tensor(np.eye(P, dtype=np.float32), name="ident128")

    groups = [list(range(NCORES))]
    AG = "AllGather"
    BYPASS = mybir.AluOpType.bypass

    with tile.TileContext(nc) as tc:
        with (
            tc.tile_pool(name="const", bufs=1) as cpool,
            tc.tile_pool(name="encp", bufs=4) as enc_pool,
            tc.tile_pool(name="small", bufs=1) as small,
            tc.tile_pool(name="psum", bufs=1, space="PSUM") as psum,
            tc.tile_pool(name="dram", bufs=1, space="DRAM") as dram,
        ):
            u_loc = dram.tile([H_SH], fp32)
            u_full = dram.tile([H], fp32, addr_space="Shared")
            e_loc = dram.tile([S_LOC], fp32)
            e_full = dram.tile([S], fp32, addr_space="Shared")

            # ---- constants ----
            # identity goes DMA -> DVE copy so the transpose matmul's two
            # inputs (mx from DVE, ident) share one wait processor.
            ident_raw = cpool.tile([P, P], fp32)
            nc.scalar.dma_start(ident_raw[:], ident_dram[:])
            ident = cpool.tile([P, P], fp32)
            nc.vector.tensor_copy(ident[:], ident_raw[:])
            ones_row = cpool.tile([1, P], fp32)  # [K=1, M=128] lhsT for bcast
            nc.vector.memset(ones_row[:], 1.0)
            ones_col = cpool.tile([P, 1], fp32)  # [K=128, M=1] lhsT for P-sum
            nc.vector.memset(ones_col[:], 1.0)

            # ---- u = hidden @ W[:, shard] on the PE ----
            wh_sb = cpool.tile([P, NO, H_SH + 1], fp32)
            nc.scalar.dma_start(wh_sb[:], wh_in[:])

            u_ps = psum.tile([1, H_SH], fp32)
            for o in range(NO):
                nc.tensor.matmul(
                    u_ps[:],
                    wh_sb[:, o, H_SH : H_SH + 1],
                    wh_sb[:, o, 0:H_SH],
                    start=(o == 0),
                    stop=(o == NO - 1),
                )
            u_sb = small.tile([1, H_SH], fp32)
            nc.scalar.copy(u_sb[:], u_ps[:])
            nc.gpsimd.dma_start(u_loc[:], u_sb[:])
            nc.gpsimd.collective_compute(
                AG, BYPASS, replica_groups=groups, ins=[u_loc[:]], outs=[u_full[:]]
            )
            # broadcast u [2048] to all 128 partitions (0-stride DMA read),
            # then a DVE copy so the stt ops' u operand is same-engine
            # (each stt instruction then carries only its enc-DMA wait).
            u_raw = cpool.tile([P, H], fp32)
            nc.gpsimd.dma_start(
                u_raw[:], u_full[:].rearrange("(a h) -> a h", a=1).broadcast_to((P, H))
            )
            u_bc = cpool.tile([P, H], fp32)
            nc.vector.tensor_copy(u_bc[:], u_raw[:])

            # ---- e_local = enc_shard @ u (fused mult + row-sum on DVE) ----
            # Tile t holds local rows {p*NT + t}, so e_sb[p, t] = e[p*NT + t]
            # and the e_loc store is per-partition contiguous.
            e_sb = small.tile([P, NT], fp32)
            scratch = small.tile([P, H], fp32)
            enc_r = enc_in.rearrange("(p n) h -> n p h", n=NT)
            for t in range(NT):
                enc_t = enc_pool.tile([P, H], fp32, tag="enc_t")
                nc.sync.dma_start(enc_t[:], enc_r[t])
                nc.vector.scalar_tensor_tensor(
                    out=scratch[:],
                    in0=enc_t[:],
                    scalar=1.0,
                    in1=u_bc[:],
                    op0=mybir.AluOpType.mult,
                    op1=mybir.AluOpType.mult,
                    accum_out=e_sb[:, t : t + 1],
                )
            nc.gpsimd.dma_start(e_loc[:].rearrange("(p n) -> p n", p=P), e_sb[:])
            nc.gpsimd.collective_compute(
                AG, BYPASS, replica_groups=groups, ins=[e_loc[:]], outs=[e_full[:]]
            )

            # ---- softmax over all 16384 energies (redundant per core) ----
            ea = small.tile([P, S // P], fp32)
            nc.gpsimd.dma_start(ea[:], e_full[:].rearrange("(p j) -> p j", p=P))
            mx = small.tile([P, 1], fp32)
            nc.vector.reduce_max(mx[:], ea[:], axis=mybir.AxisListType.X)
            # global max: transpose [128,1] -> [1,128] on PE, reduce on row 0
            mrow_ps = psum.tile([1, P], fp32)
            nc.tensor.transpose(mrow_ps[:], mx[:], ident[:])
            mrow = small.tile([1, P], fp32)
            nc.scalar.copy(mrow[:], mrow_ps[:])
            gmax = small.tile([1, 1], fp32)
            nc.vector.reduce_max(gmax[:], mrow[:], axis=mybir.AxisListType.X)
            # broadcast -gmax to [128,1]
            gb_ps = psum.tile([P, 1], fp32)
            nc.tensor.matmul(gb_ps[:], ones_row[:], gmax[:])
            nmx = small.tile([P, 1], fp32)
            nc.scalar.mul(nmx[:], gb_ps[:], -1.0)
            # exp(e - gmax) with per-partition row sums in one ACT op
            xs = small.tile([P, S // P], fp32)
            sums = small.tile([P, 1], fp32)
            nc.scalar.activation(
                xs[:],
                ea[:],
                mybir.ActivationFunctionType.Exp,
                bias=nmx[:],
                scale=1.0,
                accum_out=sums[:],
            )
            # global sum: contract the partition axis on the PE
            tot_ps = psum.tile([1, 1], fp32)
            nc.tensor.matmul(tot_ps[:], ones_col[:], sums[:])
            tot = small.tile([1, 1], fp32)
            nc.scalar.copy(tot[:], tot_ps[:])
            rec = small.tile([1, 1], fp32)
            nc.vector.reciprocal(rec[:], tot[:])
            rb_ps = psum.tile([P, 1], fp32)
            nc.tensor.matmul(rb_ps[:], ones_row[:], rec[:])
            rec_bc = small.tile([P, 1], fp32)
            nc.scalar.copy(rec_bc[:], rb_ps[:])
            outx = small.tile([P, S // P], fp32)
            nc.vector.tensor_scalar_mul(outx[:], xs[:], rec_bc[:])
            nc.sync.dma_start(attn_out.rearrange("(p j) -> p j", p=P), outx[:])

    nc.compile()
    return nc


def _get_program():
    if "nc" not in _CACHE:
        _CACHE["nc"] = _build_program()
    return _CACHE["nc"]


def _make_in_maps(hidden, encoder_outputs, W):
    hidden = np.ascontiguousarray(np.asarray(hidden, dtype=np.float32))
    enc = np.ascontiguousarray(np.asarray(encoder_outputs, dtype=np.float32))
    W = np.ascontiguousarray(np.asarray(W, dtype=np.float32))
    hid = hidden.reshape(NO, P).transpose(1, 0)  # hid[p, o] = hidden[o*128+p]
    # W as [p, o, h]: W_poh[p, o, h] = W[o*128+p, h]
    W_poh = W.reshape(NO, P, H).transpose(1, 0, 2)
    in_maps = []
    for r in range(NCORES):
        wh = np.empty((P, NO, H_SH + 1), dtype=np.float32)
        wh[:, :, 0:H_SH] = W_poh[:, :, r * H_SH : (r + 1) * H_SH]
        wh[:, :, H_SH] = hid
        in_maps.append(
            {
                "enc": np.ascontiguousarray(enc[r * S_LOC : (r + 1) * S_LOC]),
                "wh": wh,
            }
        )
    return in_maps


def run(hidden, encoder_outputs, W, b=None, trace=False):
    from concourse.bass_utils import run_bass_kernel_spmd

    nc = _get_program()
    in_maps = _make_in_maps(hidden, encoder_outputs, W)
    res = run_bass_kernel_spmd(nc, in_maps, list(range(NCORES)), trace=trace)
    out = np.asarray(res.results[0]["attn"], dtype=np.float32).reshape(1, 1, S)
    return out, res


def kernel(hidden, encoder_outputs, W, b):
    out, _ = run(hidden, encoder_outputs, W, b)
    return out
